# revision 1
# baseline (speedup 1.0000x reference)
"""Trainium2 Bass kernel for nn_EquAttentionGATv2 (gnn_message_passing), v2.

Differences from the v1 baseline (which shipped per-edge q_src AND q_dst and
did all SO(3) linears per-edge):
  - g_r is computed per-NODE on device (10 small matmul blocks per core) and
    delivered to edges by a one-hot gather matmul that ACCUMULATES directly
    into the same PSUM region as the per-edge g_l_src pair matmuls.  This
    halves the per-edge DMA stream (only q_src is shipped) and removes 10 of
    the 15 per-group PE matmuls.
  - The separate per-edge g_l copy is eliminated via the softmax identity
        out = sum_e attn*(g_l+g_r) - g_r * sum_e attn = sum_e attn*g_l
    (sum attn = 1 per node), so the scatter accumulates attn-weighted gsum
    and g_r is subtracted once per node at the end.
  - One-hot matrices (scatter s01 with envelope folded, and gather s01gT)
    are host-built and streamed, freeing DVE cycles.
  - exp(z) = (1+tanh(z/2))/(1-tanh(z/2)) stays on the silu ACT table;
    the division uses the DVE reciprocal.
  - Output rows are (s, h)-contiguous (natural layout), no host transpose.
"""

import numpy as np

import concourse.bass as bass
import concourse.mybir as mybir
from concourse.tile import TileContext
from concourse import bass_utils

# ----------------------------------------------------------------------------
# problem constants (hardcoded; kernel must be self-contained)
# ----------------------------------------------------------------------------
N_NODES = 10000
N_EDGES = 160000
S = 9            # (lmax+1)^2 spherical harmonic coeffs
C_IN = 64
H = 64
N_CORES = 8
NPC = 1250       # nodes per core
NBLK = 10        # 128-node blocks per core
BN = 128         # block node count
NPAD = NBLK * BN  # 1280 padded nodes per core
SH = S * H       # 576
L_OF_S = [0, 1, 1, 1, 2, 2, 2, 2, 2]
PAIRS = [(0, 1), (2, 3), (4, 5), (6, 7), (8, None)]
NP_ = len(PAIRS)          # 5
TILE_E = 1024             # edges per packed DMA chunk
GE = 128                  # edges per compute group
SCATTER_LAG = 6           # groups the PE scatter trails the compute chain

F16 = mybir.dt.float16
F32 = mybir.dt.float32


# ----------------------------------------------------------------------------
# workaround: this container's walrus rejects >1 semaphore wait per
# instruction.  Hoist extra waits onto same-engine NOPs.
# ----------------------------------------------------------------------------
def _split_multi_waits(nc, max_waits=1):
    for f in nc.m.functions:
        for bb in f.blocks:
            out = []
            for inst in list(bb.instructions):
                si = inst.sync_info
                if si is not None and len(si.on_wait) > max_waits:
                    waits = list(si.on_wait)
                    extra, keep = waits[:-max_waits], waits[-max_waits:]
                    for w in extra:
                        out.append(
                            mybir.InstNoOp(
                                name=nc.get_next_instruction_name(),
                                sync_info=mybir.SyncInfo(on_wait=[w], on_update=[]),
                                bass_nofuse=True,
                                engine=inst.engine,
                            )
                        )
                    si.on_wait[:] = keep
                out.append(inst)
            bb.instructions = out


def _ap(apview, free_dims, extra_offset=0):
    """AP with the partition dim of `apview` and custom free dims
    [(stride, count), ...] in elements."""
    lst = [list(apview.ap[0])] + [[s, c] for s, c in free_dims]
    return bass.AP(apview.tensor, apview.offset + extra_offset, lst)


# ----------------------------------------------------------------------------
# device program
# ----------------------------------------------------------------------------
def _build_nc(b_e, has_bias, repeat=1):
    """b_e: edges per 128-node block (multiple of GE).
    repeat: run the whole compute body N times (benchmarking builds only --
    the repeat-difference of wall times isolates per-execution device time
    from dispatch/transfer overhead)."""
    e_dev = NBLK * b_e
    e_chunks = -(-e_dev // TILE_E) * TILE_E
    nchunks = e_chunks // TILE_E
    gpb = b_e // GE                   # groups per block
    spc = TILE_E // 128               # scatter slots per chunk
    # packed per-chunk layout (fp16 elems): [ qe 5*TILE_E | s01s spc*128 | s01gt TILE_E ]
    CW = NP_ * TILE_E + spc * 128 + TILE_E
    OFF_S01 = NP_ * TILE_E
    OFF_GT = NP_ * TILE_E + spc * 128

    nc = bass.Bass()

    pk = nc.dram_tensor("pk", [128, nchunks, CW], F16, kind="ExternalInput")
    qn = nc.dram_tensor("qn", [128, NP_, NPAD], F16, kind="ExternalInput")
    w2 = nc.dram_tensor("w2", [128, 2 * NP_, 128], F16, kind="ExternalInput")
    wrep = nc.dram_tensor("wrep", [128, SH], F16, kind="ExternalInput")
    if has_bias:
        brep = nc.dram_tensor("brep", [128, 2 * H], F32, kind="ExternalInput")
    outd = nc.dram_tensor("outd", [NPAD, SH], F32, kind="ExternalOutput")

    AF = mybir.ActivationFunctionType
    OP = mybir.AluOpType

    with TileContext(nc) as tc:
        with (
            tc.tile_pool(name="const", bufs=1) as constp,
            tc.tile_pool(name="qe", bufs=5) as qep,
            tc.tile_pool(name="sil", bufs=4) as silp,
            tc.tile_pool(name="rhs", bufs=SCATTER_LAG + 3) as rhsp,
            tc.tile_pool(name="blk", bufs=2) as blkp,
            tc.tile_pool(name="sm", bufs=3) as smp,
            tc.tile_pool(name="outn", bufs=2) as outp,
            tc.tile_pool(name="psg", bufs=3, space="PSUM") as psgp,
            tc.tile_pool(name="pso", bufs=1, space="PSUM") as psop,
        ):
            # ---------------- constants ----------------
            w2_sb = constp.tile([128, 2 * NP_ * 128], F16)
            nc.sync.dma_start(w2_sb[:], w2[:].rearrange("p a b -> p (a b)"))
            w2v = w2_sb[:].rearrange("p (a b) -> p a b", b=128)
            wrep_sb = constp.tile([128, SH], F16)
            nc.sync.dma_start(wrep_sb[:], wrep[:])
            qn_sb = constp.tile([128, NP_ * NPAD], F16)
            nc.sync.dma_start(
                qn_sb[:].rearrange("p (a n) -> p a n", n=NPAD), qn[:]
            )
            qnv = qn_sb[:].rearrange("p (a n) -> p a n", n=NPAD)
            if has_bias:
                brep_sb = constp.tile([128, 2 * H], F32)
                nc.sync.dma_start(brep_sb[:], brep[:])
            # g_r for all local nodes, node-major (s,h) rows: [128, NBLK, 640]
            gr_sb = constp.tile([128, NBLK * 640], F16)
            grv = gr_sb[:].rearrange("p (b f) -> p b f", f=640)

            # ---------------- per-node g_r ----------------
            for _rep in range(repeat):
              for nb in range(NBLK):
                  psn = psgp.tile([128, 1024], F32, tag="ps")
                  for p in range(NP_):
                      nc.tensor.matmul(
                          psn[:, p * 128:(p + 1) * 128],
                          lhsT=qnv[:, p, nb * BN:(nb + 1) * BN],
                          rhs=w2v[:, NP_ + p, :],
                          start=True, stop=True, skip_group_check=True,
                      )
                  if has_bias:
                      nc.vector.tensor_tensor(
                          psn[:, 0:H], psn[:, 0:H], brep_sb[:, H:2 * H], OP.add
                      )
                  nc.scalar.activation(grv[:, nb, :], psn[:, 0:640], AF.Copy)

              # ---------------- edge phase ----------------
              pk_tiles = {}

              def get_chunk(ci):
                  # keep a sliding window of live chunk tiles: a block's loop1
                  # and loop2 both walk the same 2-3 chunks, and adjacent
                  # blocks share a boundary chunk.
                  if ci not in pk_tiles:
                      t = qep.tile([128, CW], F16, tag="pk")
                      nc.sync.dma_start(t[:], pk[:, ci, :])
                      for k in [k for k in pk_tiles if k < ci - 3]:
                          del pk_tiles[k]
                      pk_tiles[ci] = t
                  return pk_tiles[ci]

              pending = []

              def run_pending(force=False):
                  while pending and (force or len(pending) > SCATTER_LAG):
                      pending.pop(0)()

              for b in range(NBLK):
                  ps_out = psop.tile([128, 1024], F32)
                  for gb in range(gpb):
                      e0 = b * b_e + gb * GE
                      ci, eo = divmod(e0, TILE_E)
                      t = get_chunk(ci)
                      qv = t[:, 0:NP_ * TILE_E].rearrange(
                          "p (a e) -> p a e", e=TILE_E
                      )
                      gt = t[:, OFF_GT:OFF_GT + TILE_E]
                      slot = eo // 128

                      ps = psgp.tile([128, 1024], F32, tag="ps")
                      # one-hot gather of g_r[dst] (start clears banks),
                      # then g_l_src pair matmuls accumulate on top.
                      nc.tensor.matmul(
                          ps[:, 0:512], lhsT=gt[:, eo:eo + GE],
                          rhs=grv[:, b, 0:512],
                          start=True, stop=False, skip_group_check=True,
                      )
                      nc.tensor.matmul(
                          ps[:, 512:576], lhsT=gt[:, eo:eo + GE],
                          rhs=grv[:, b, 512:SH],
                          start=True, stop=False, skip_group_check=True,
                      )
                      for p in range(NP_ - 1):
                          nc.tensor.matmul(
                              ps[:, p * 128:(p + 1) * 128],
                              lhsT=qv[:, p, eo:eo + GE], rhs=w2v[:, p, :],
                              start=False, stop=True, skip_group_check=True,
                          )
                      nc.tensor.matmul(
                          ps[:, 512:640],
                          lhsT=qv[:, NP_ - 1, eo:eo + GE],
                          rhs=w2v[:, NP_ - 1, :],
                          start=False, stop=True, skip_group_check=True,
                      )
                      if has_bias:
                          nc.vector.tensor_tensor(
                              ps[:, 0:H], ps[:, 0:H], brep_sb[:, 0:H], OP.add
                          )

                      # silu(gsum); raw gsum stays in PSUM until the P-mult
                      sil = silp.tile([128, SH], F16, tag="sil")
                      nc.scalar.activation(sil[:], ps[:, 0:SH], AF.Silu)

                      # logits z[e,s] = <silu, attn_w>: fp16 mult then
                      # fp16-accumulated reduce over h (|z| < ~10, fine)
                      nc.vector.tensor_tensor(sil[:], sil[:], wrep_sb[:], OP.mult)
                      zt = smp.tile([128, S], F16, tag="zt")
                      with nc.allow_low_precision(
                          reason="logit partial sums in fp16; |z|<~10 so "
                          "absolute error ~1e-2 max, well inside rel tol"
                      ):
                          nc.vector.tensor_reduce(
                              zt[:],
                              sil[:].rearrange("p (s h) -> p s h", h=H),
                              mybir.AxisListType.X, OP.add,
                          )
                      # ee = exp(z) = (1+t)/(1-t) = 2/(1-t) - 1, t = tanh(z/2)
                      th = smp.tile([128, S], F32, tag="th")
                      nc.scalar.activation(th[:], zt[:], AF.Tanh, scale=0.5)
                      bb_ = smp.tile([128, S], F32, tag="bb")
                      nc.vector.tensor_scalar(
                          bb_[:], th[:], 1.0, -1.0, OP.subtract, OP.mult
                      )
                      rr = smp.tile([128, S], F32, tag="rr")
                      nc.vector.reciprocal(rr[:], bb_[:])
                      rhs = rhsp.tile([128, 592], F16)
                      nc.vector.tensor_scalar(
                          rhs[:, SH:SH + S], rr[:], 2.0, 1.0, OP.mult, OP.subtract
                      )
                      # P = ee * gsum, read straight from PSUM.  The PSUM
                      # f32 operand forces 1x mode anyway, so ee is applied
                      # as a plain step-0 broadcast (no pair expansion).
                      nc.vector.tensor_tensor(
                          _ap(rhs[:], [(64, S), (1, 64)]),
                          _ap(ps[:, 0:SH], [(64, S), (1, 64)]),
                          _ap(rhs[:, SH:SH + S], [(1, S), (0, 64)]),
                          OP.mult,
                      )

                      def scat(ps_out=ps_out, t=t, slot=slot, rhs=rhs,
                               first=(gb == 0), last=(gb == gpb - 1)):
                          sv = t[:, OFF_S01:OFF_S01 + spc * 128].rearrange(
                              "p (c m) -> p c m", m=128
                          )
                          nc.tensor.matmul(
                              ps_out[:, 0:512], lhsT=sv[:, slot, :],
                              rhs=rhs[:, 0:512],
                              start=first, stop=last, skip_group_check=True,
                          )
                          nc.tensor.matmul(
                              ps_out[:, 512:SH + S], lhsT=sv[:, slot, :],
                              rhs=rhs[:, 512:SH + S],
                              start=first, stop=last, skip_group_check=True,
                          )

                      pending.append(scat)
                      run_pending()

                  def norm(ps_out=ps_out, b=b):
                      den = smp.tile([128, S], F32, tag="den")
                      nc.vector.tensor_scalar_max(den[:], ps_out[:, SH:SH + S], 1e-30)
                      rec = smp.tile([128, S], F32, tag="rec")
                      nc.vector.reciprocal(rec[:], den[:])
                      on = outp.tile([128, SH], F32, tag="on")
                      # out = ps_out * (1/den) - g_r
                      nc.vector.tensor_tensor(
                          _ap(on[:], [(64, S), (1, 64)]),
                          _ap(ps_out[:, 0:SH], [(64, S), (1, 64)]),
                          _ap(rec[:], [(1, S), (0, 64)]),
                          OP.mult,
                      )
                      nc.vector.tensor_tensor(
                          on[:], on[:], grv[:, b, 0:SH], OP.subtract
                      )
                      nc.sync.dma_start(outd[b * BN:(b + 1) * BN, :], on[:])

                  pending.append(norm)

              run_pending(force=True)

    _split_multi_waits(nc)
    return nc


# ----------------------------------------------------------------------------
# host-side sharding / input prep
# ----------------------------------------------------------------------------
def _prepare(q, envelope, edge_index, w_l, b_l, w_r, b_r, attn_w):
    q = np.asarray(q, dtype=np.float32)
    env = np.asarray(envelope, dtype=np.float32)
    ei = np.asarray(edge_index).astype(np.int64)
    src, dst = ei[0], ei[1]

    order = np.argsort(dst, kind="stable")
    src_s, dst_s, env_s = src[order], dst[order], env[order]
    core_of = dst_s // NPC

    blk_of = (dst_s - core_of * NPC) // BN
    counts = np.zeros((N_CORES, NBLK), dtype=np.int64)
    np.add.at(counts, (core_of, blk_of), 1)
    b_e = int(np.ceil(counts.max() / GE) * GE)
    e_dev = NBLK * b_e
    e_chunks = int(np.ceil(e_dev / TILE_E) * TILE_E)
    nchunks = e_chunks // TILE_E
    spc = TILE_E // 128
    CW = NP_ * TILE_E + spc * 128 + TILE_E
    OFF_S01 = NP_ * TILE_E
    OFF_GT = NP_ * TILE_E + spc * 128

    # stacked-transposed q: qT2[pair, 64*i + c, n] = q[n, s_{2p+i}, c]
    qT2 = np.zeros((NP_, 128, N_NODES), dtype=np.float16)
    for p, (sa, sb) in enumerate(PAIRS):
        qT2[p, 0:64, :] = q[:, sa, :].T
        if sb is not None:
            qT2[p, 64:128, :] = q[:, sb, :].T

    def w2_of(w):
        w = np.asarray(w, dtype=np.float32)
        out = np.zeros((NP_, 128, 128), dtype=np.float16)
        for p, (sa, sb) in enumerate(PAIRS):
            out[p, 0:64, 0:64] = w[L_OF_S[sa]].T
            if sb is not None:
                out[p, 64:128, 64:128] = w[L_OF_S[sb]].T
        return out

    # planes 0..4 = W2_l (edge gsum), 5..9 = W2_r (per-node g_r)
    w2_dev = np.concatenate([w2_of(w_l), w2_of(w_r)], axis=0)
    w2_dev = np.ascontiguousarray(w2_dev.transpose(1, 0, 2))

    wrep = np.tile(np.tile(np.asarray(attn_w, np.float32), S)[None, :], (128, 1))
    wrep = wrep.astype(np.float16)

    b_l = np.asarray(b_l, np.float32)
    b_r = np.asarray(b_r, np.float32)
    has_bias = bool(np.any(b_l) or np.any(b_r))
    brep = None
    if has_bias:
        brep = np.tile(
            np.concatenate([b_l, b_r])[None, :], (128, 1)
        ).astype(np.float32)

    # in-degree for isolated-node fixup (out = -g_r there without it)
    indeg = np.zeros(N_NODES, dtype=np.int64)
    np.add.at(indeg, dst_s, 1)

    in_maps = []
    for c in range(N_CORES):
        m = core_of == c
        sc, dc, ec = src_s[m], dst_s[m] - c * NPC, env_s[m]
        bc_ = dc // BN

        src_pad = np.zeros(e_dev, dtype=np.int64)
        dloc = np.full(e_dev, -1, dtype=np.int64)   # dst slot within block
        env_pad = np.ones(e_dev, dtype=np.float32)

        starts = np.searchsorted(bc_, np.arange(NBLK))
        ends = np.searchsorted(bc_, np.arange(NBLK), side="right")
        for b in range(NBLK):
            s0, s1 = starts[b], ends[b]
            n = s1 - s0
            pos = b * b_e + np.arange(n)
            src_pad[pos] = sc[s0:s1]
            dloc[pos] = dc[s0:s1] - b * BN
            env_pad[pos] = ec[s0:s1]

        pk_dev = np.zeros((128, nchunks, CW), dtype=np.float16)
        qe_view = pk_dev[:, :, :NP_ * TILE_E].reshape(
            128, nchunks, NP_, TILE_E
        )
        qe_flat = np.zeros((128, NP_, e_chunks), dtype=np.float16)
        qe_flat[:, :, :e_dev] = qT2[:, :, src_pad].transpose(1, 0, 2)
        qe_view[:] = qe_flat.reshape(128, NP_, nchunks, TILE_E).transpose(
            0, 2, 1, 3
        )

        # local node q (pad nodes beyond NPC are zero)
        qn_dev = np.zeros((128, NP_, NPAD), dtype=np.float16)
        nids = np.arange(c * NPC, (c + 1) * NPC)
        qn_dev[:, :, :NPC] = qT2[:, :, nids].transpose(1, 0, 2)

        real = dloc >= 0
        eidx = np.arange(e_dev)
        # scatter one-hot with envelope folded: [e, m] edge-major
        s01 = np.zeros((e_chunks, 128), dtype=np.float16)
        s01[eidx[real], dloc[real]] = (env_pad[real] + 1e-7).astype(np.float16)
        # [e, m] -> [p=e%128, chunk, slot=(e//128)%spc, m]
        pk_dev[:, :, OFF_S01:OFF_GT] = np.ascontiguousarray(
            s01.reshape(nchunks, spc, 128, 128).transpose(2, 0, 1, 3)
        ).reshape(128, nchunks, spc * 128)
        # gather one-hot transposed: [m, e]
        s01gt_dev = np.zeros((128, e_chunks), dtype=np.float16)
        s01gt_dev[dloc[real], eidx[real]] = 1.0
        pk_dev[:, :, OFF_GT:] = s01gt_dev.reshape(128, nchunks, TILE_E)

        im = {
            "pk": pk_dev,
            "qn": qn_dev,
            "w2": w2_dev,
            "wrep": wrep,
        }
        if has_bias:
            im["brep"] = brep
        in_maps.append(im)

    return b_e, has_bias, in_maps, indeg


# ----------------------------------------------------------------------------
# cached compile + PJRT runner (same machinery as v1)
# ----------------------------------------------------------------------------
_CACHE = {}
LAST_BENCH_NS = None


def _get_runner(b_e, has_bias, repeat=1):
    key = (b_e, has_bias, repeat)
    if key in _CACHE:
        return _CACHE[key]
    runner = _make_runner(_build_nc(b_e, has_bias, repeat))
    _CACHE[key] = runner
    return runner


def _make_runner(nc):
    import jax
    from jax.sharding import Mesh, PartitionSpec
    from jax.experimental.shard_map import shard_map
    from concourse import bass2jax

    bass2jax.install_neuronx_cc_hook()

    in_names, out_names, out_avals, zero_outs = [], [], [], []
    partition_name = nc.partition_id_tensor.name if nc.partition_id_tensor else None
    for alloc in nc.m.functions[0].allocations:
        if not isinstance(alloc, mybir.MemoryLocationSet):
            continue
        name = alloc.memorylocations[0].name
        if alloc.kind == "ExternalInput":
            if name != partition_name:
                in_names.append(name)
        elif alloc.kind == "ExternalOutput":
            shape = tuple(alloc.tensor_shape)
            dtype = mybir.dt.np(alloc.dtype)
            out_names.append(name)
            out_avals.append(jax.core.ShapedArray(shape, dtype))
            zero_outs.append(np.zeros(shape, dtype))
    n_params = len(in_names)
    n_outs = len(out_avals)
    all_in_names = list(in_names) + list(out_names)
    if partition_name is not None:
        all_in_names.append(partition_name)

    def _body(*args):
        operands = list(args)
        if partition_name is not None:
            operands.append(bass2jax.partition_id_tensor())
        outs = bass2jax._bass_exec_p.bind(
            *operands,
            out_avals=tuple(out_avals),
            in_names=tuple(all_in_names),
            out_names=tuple(out_names),
            lowering_input_output_aliases=(),
            sim_require_finite=True,
            sim_require_nnan=True,
            nc=nc,
        )
        return tuple(outs)

    def _chain_body(kk):
        def _chain(*args):
            ins = list(args[:n_params])
            outs = list(args[n_params:])
            for _ in range(kk):
                operands = list(ins) + list(outs)
                if partition_name is not None:
                    operands.append(bass2jax.partition_id_tensor())
                outs = list(bass2jax._bass_exec_p.bind(
                    *operands,
                    out_avals=tuple(out_avals),
                    in_names=tuple(all_in_names),
                    out_names=tuple(out_names),
                    lowering_input_output_aliases=(),
                    sim_require_finite=True,
                    sim_require_nnan=True,
                    nc=nc,
                ))
            return tuple(outs)
        return _chain

    devices = jax.devices()[:N_CORES]
    mesh = Mesh(np.asarray(devices), ("core",))
    in_specs = (PartitionSpec("core"),) * (n_params + n_outs)
    out_specs = (PartitionSpec("core"),) * n_outs
    donate = tuple(range(n_params, n_params + n_outs))
    sharded = jax.jit(
        shard_map(_body, mesh=mesh, in_specs=in_specs, out_specs=out_specs,
                  check_rep=False),
        donate_argnums=donate,
        keep_unused=True,
    )

    _chain_cache = {}

    def get_chain(kk):
        if kk not in _chain_cache:
            _chain_cache[kk] = jax.jit(
                shard_map(_chain_body(kk), mesh=mesh, in_specs=in_specs,
                          out_specs=out_specs, check_rep=False),
                donate_argnums=donate,
                keep_unused=True,
            )
        return _chain_cache[kk]

    return {
        "fn": sharded,
        "get_chain": get_chain,
        "in_names": in_names,
        "out_names": out_names,
        "out_avals": out_avals,
        "zero_outs": zero_outs,
        "mesh": mesh,
    }


def _bench_runner(b_e, has_bias, concat_in, n, repeat=4):
    """Per-exec device time via the repeat-difference method: the same
    kernel is built with the compute body unrolled 1x and Rx; each is run
    as a single dispatch and T = (wall_R - wall_1)/(R - 1).  Dispatch and
    host<->device transfer overhead cancels in the difference."""
    import time
    import jax
    from jax.sharding import NamedSharding, PartitionSpec

    r1 = _get_runner(b_e, has_bias, 1)
    rR = _get_runner(b_e, has_bias, repeat)
    sh = NamedSharding(r1["mesh"], PartitionSpec("core"))
    dev_in = [jax.device_put(a, sh) for a in concat_in]
    jax.block_until_ready(dev_in)

    def zs(r):
        return [
            jax.device_put(
                np.zeros((N_CORES * z.shape[0], *z.shape[1:]), z.dtype), sh
            )
            for z in r["zero_outs"]
        ]

    def run(r):
        bufs = zs(r)
        t0 = time.perf_counter()
        outs = r["fn"](*dev_in, *bufs)
        jax.block_until_ready(outs)
        return time.perf_counter() - t0

    run(r1); run(rR)  # warmup
    # The axon dispatch wall is 60-160 ms with tens-of-ms drift, far above
    # the few-ms device-time signal.  Take many interleaved pairs so drift
    # is shared within a pair; the median of paired diffs is the primary
    # estimate, with a rank-matched minimum as fallback.
    w1s, wRs = [], []
    for _ in range(max(110, n)):
        w1s.append(run(r1))
        wRs.append(run(rR))
    diffs = sorted((b - a) / (repeat - 1) for a, b in zip(w1s, wRs))
    est = diffs[len(diffs) // 2]
    if est <= 0:
        est = (min(wRs) - min(w1s)) / (repeat - 1)
    return max(est, 1e-6) * 1e9


_TRIVIAL = {}


def bench_overhead(n=10):
    if "r" not in _TRIVIAL:
        nc = bass.Bass()
        x = nc.dram_tensor("x", [128, 128], F32, kind="ExternalInput")
        y = nc.dram_tensor("y", [128, 128], F32, kind="ExternalOutput")
        with TileContext(nc) as tc:
            with tc.tile_pool(name="p", bufs=1) as pool:
                t = pool.tile([128, 128], F32)
                nc.sync.dma_start(t[:], x[:])
                nc.vector.tensor_scalar_mul(t[:], t[:], 1.0)
                nc.sync.dma_start(y[:], t[:])
        _split_multi_waits(nc)
        _TRIVIAL["r"] = _make_runner(nc)
    r = _TRIVIAL["r"]
    xin = np.zeros((N_CORES * 128, 128), np.float32)
    return _bench_runner(r, [xin], n)


def kernel(q, k, v, envelope, edge_index, w_l, b_l, w_r, b_r, attn_w,
           _bench=0):
    global LAST_BENCH_NS
    b_e, has_bias, in_maps, indeg = _prepare(
        q, envelope, edge_index, w_l, b_l, w_r, b_r, attn_w
    )
    r = _get_runner(b_e, has_bias)

    concat_in = [
        np.concatenate([im[name] for im in in_maps], axis=0)
        for name in r["in_names"]
    ]

    zeros = [
        np.zeros((N_CORES * z.shape[0], *z.shape[1:]), z.dtype)
        for z in r["zero_outs"]
    ]
    outs = [np.asarray(o) for o in r["fn"](*concat_in, *zeros)]

    if _bench:
        LAST_BENCH_NS = _bench_runner(b_e, has_bias, concat_in, _bench)

    full = outs[0].reshape(N_CORES, NPAD, SH)
    out = np.concatenate([full[c, :NPC] for c in range(N_CORES)], axis=0)
    out = out.reshape(N_NODES, S, H)
    if (indeg == 0).any():
        out[indeg == 0] = 0.0
    return np.ascontiguousarray(out, dtype=np.float32)

